# revision 7
# baseline (speedup 1.0000x reference)
"""Trainium2 Bass kernel for the AttentionLstm problem.

Strategy: data-parallel over batch (8 batches per NeuronCore, 8 cores).
Single NEFF launch per call:
  Phase A (precompute): mean/h0/c0, we_proj = word_embeds @ W_ihE.T + bias
      (via DRAM roundtrip), a1 = feat @ Wa.T (transposed layout),
      fproj = feat @ W_ihF.T ((b,l)-on-partition for the context matmul).
  Phase B (recurrence, 20 steps): attention scores in transposed layout,
      masked softmax on 8 partitions, context fused into the gates PSUM
      accumulation as a block-diagonal matmul with the softmax weights,
      LSTM tail with tanh-only activations (sigmoid(x) = 0.5*tanh(x/2)+0.5
      algebra folded into scalar_tensor_tensor ops; state kept as S=2c,
      H'=2h with weights pre-scaled on host).
  Phase C: logits = H' @ (0.5*fc_W).T + fc_b streaming fc_W bf16 from HBM
      (prefetched into SBUF during phases A/B).

All matmul operands are bf16 (host-cast); accumulation fp32 in PSUM.
"""

from contextlib import ExitStack

import ml_dtypes
import numpy as np

import concourse.bass as bass  # noqa: F401
import concourse.tile as tile
from concourse import bacc, mybir

BF16 = ml_dtypes.bfloat16
mdt = mybir.dt
AF = mybir.ActivationFunctionType
ALU = mybir.AluOpType
AX = mybir.AxisListType

B, T, L, F, E, H, V = 64, 20, 49, 2048, 512, 512, 30000
NCORES = 8
J = B // NCORES          # 8 batches per core
BL = J * L               # 392
TOK = T * J              # 160
FKT = F // 128           # 16
HKT = H // 128           # 4
G4 = 4 * H               # 2048 (gate width, reordered [i, f, o, g])
NCH = G4 // 512          # 4 psum chunks of 512
VCH = (V + 511) // 512   # 59 vocab chunks


def _chunks_v():
    return [(v * 512, min(512, V - v * 512)) for v in range(VCH)]


def build_ir(ctx: ExitStack, tc: tile.TileContext, d: dict):
    """Emit the kernel IR. `d` maps names -> DRAM APs."""
    nc = tc.nc

    featT = d["featT"]      # [2048, 392] bf16
    weT = d["weT"]          # [512, 160] bf16
    WaT = d["WaT"]          # [2048, 512] bf16
    WihFT = d["WihFT"]      # [2048, 2048] bf16
    WiheT = d["WiheT"]      # [513, 2048] bf16 (row 512 = gate bias)
    WbigT = d["WbigT"]      # [512, 2048] bf16
    UaT = d["UaT"]          # [512, 512] bf16
    ihcWT = d["ihcWT"]      # [2048, 1024] bf16
    ihcB = d["ihcB"]        # [1, 1024] bf16
    baR = d["baR"]          # [1, 512] bf16
    vaRep = d["vaRep"]      # [512, 8] bf16
    fcWT = d["fcWT"]        # [512, 30000] bf16
    fcB = d["fcB"]          # [1, 30000] bf16
    maskD = d["maskD"]      # [8, 392] f32
    outsD = d["outs"]       # [8, 20, 30000] f32 out
    wsD = d["ws"]           # [8, 20, 49] f32 out

    # ---------------- persistent pools ----------------
    P = ctx.enter_context(tc.tile_pool(name="persist", bufs=1))
    fcpool = ctx.enter_context(tc.tile_pool(name="fcw", bufs=6))
    dram = ctx.enter_context(tc.tile_pool(name="dram", bufs=1, space="DRAM"))

    eye_np = np.zeros((8, 8), np.float32)
    np.fill_diagonal(eye_np, 1.0)
    eyeD = nc.inline_tensor(eye_np.astype(BF16), name="eye8")
    eye8 = P.tile([8, 8], mdt.bfloat16, tag="eye8")
    nc.sync.dma_start(eye8[:], eyeD.ap())

    ones_row = P.tile([1, TOK], mdt.bfloat16, tag="ones_row")
    nc.vector.memset(ones_row[:], 1.0)
    ones392 = P.tile([1, BL], mdt.bfloat16, tag="ones392")
    nc.vector.memset(ones392[:], 1.0)

    mask_sb = P.tile([J, BL], mdt.float32, tag="mask")
    nc.sync.dma_start(mask_sb[:], maskD[:])

    vaRep_sb = P.tile([128, HKT * J], mdt.bfloat16, tag="vaRep")
    nc.sync.dma_start(vaRep_sb[:].rearrange("p (k m) -> p k m", k=HKT),
                      vaRep.rearrange("(k p) m -> p k m", p=128))

    WbigT_sb = P.tile([128, HKT * G4], mdt.bfloat16, tag="WbigT")
    nc.sync.dma_start(WbigT_sb[:].rearrange("p (k g) -> p k g", k=HKT),
                      WbigT.rearrange("(k p) g -> p k g", p=128))

    UaT_sb = P.tile([128, HKT * H], mdt.bfloat16, tag="UaT")
    nc.sync.dma_start(UaT_sb[:].rearrange("p (k h) -> p k h", k=HKT),
                      UaT.rearrange("(k p) h -> p k h", p=128))

    baR_sb = P.tile([1, H], mdt.bfloat16, tag="baR")
    nc.sync.dma_start(baR_sb[:], baR[:])

    # phase-B persistent state
    a1T_sb = P.tile([128, HKT * BL], mdt.bfloat16, tag="a1T")
    fproj_sb = [P.tile([128, G4], mdt.bfloat16, tag=f"fproj{q}",
                       name=f"fproj{q}") for q in range(4)]
    HT = P.tile([128, HKT * TOK], mdt.bfloat16, tag="HT")
    hT0_sb = P.tile([128, HKT * J], mdt.bfloat16, tag="hT0")
    S_sb = [P.tile([J, H], mdt.float32, tag=f"S{i}", name=f"S{i}")
            for i in range(2)]
    ws_sb = P.tile([J, T * L], mdt.float32, tag="ws_sb")
    weproj_dram = dram.tile([TOK, G4], mdt.bfloat16)

    # prefetch fc weight chunks (no deps -> DMAs start immediately,
    # pool slots throttle how far ahead we run)
    fcw_tiles = []
    for v, (v0, vn) in enumerate(_chunks_v()):
        ft = fcpool.tile([128, HKT, 512], mdt.bfloat16, tag="fcw")
        nc.sync.dma_start(ft[:, :, 0:vn],
                          fcWT[:, v0:v0 + vn].rearrange("(k p) n -> p k n", p=128))
        fcw_tiles.append(ft)

    # ---------------- phase A ----------------
    with tc.tile_pool(name="phAf", bufs=1) as PAF, \
         tc.tile_pool(name="psA_big", bufs=1, space="PSUM") as PSB, \
         tc.tile_pool(name="psA_small", bufs=1, space="PSUM") as PSM:

        featT_sb = PAF.tile([128, FKT * BL], mdt.bfloat16, tag="featT")
        nc.sync.dma_start(featT_sb[:].rearrange("p (k c) -> p k c", k=FKT),
                          featT.rearrange("(k p) c -> p k c", p=128))

        # --- phase A1: mean/h0/c0 and we_proj ---
        with tc.tile_pool(name="phA1", bufs=1) as PA, \
             tc.tile_pool(name="phA1_ws", bufs=3) as WS:

            WiheT_sb = PA.tile([128, HKT * G4], mdt.bfloat16, tag="WiheT")
            nc.sync.dma_start(WiheT_sb[:].rearrange("p (k g) -> p k g", k=HKT),
                              WiheT[0:512, :].rearrange("(k p) g -> p k g", p=128))
            wihe_brow = PA.tile([1, G4], mdt.bfloat16, tag="wihe_brow")
            nc.sync.dma_start(wihe_brow[:], WiheT[512:513, :])

            weT_sb = PA.tile([128, HKT * TOK], mdt.bfloat16, tag="weT")
            nc.sync.dma_start(weT_sb[:].rearrange("p (k m) -> p k m", k=HKT),
                              weT.rearrange("(k p) m -> p k m", p=128))

            ihcB_sb = PA.tile([1, 1024], mdt.bfloat16, tag="ihcB")
            nc.sync.dma_start(ihcB_sb[:], ihcB[:])

            # mean over l: featT [p, (k j l)] -> sum_l * (1/49)
            meanT_f = PA.tile([128, FKT * J], mdt.float32, tag="meanT_f")
            nc.vector.tensor_reduce(
                meanT_f[:].rearrange("p (k j) -> p k j", k=FKT),
                featT_sb[:].rearrange("p (k j l) -> p k j l", k=FKT, j=J),
                axis=AX.X, op=ALU.add)
            meanT = PA.tile([128, FKT * J], mdt.bfloat16, tag="meanT")
            nc.vector.tensor_scalar_mul(meanT[:], meanT_f[:], 1.0 / L)

            # hc0 = mean @ [2*ih_W; 2*ic_W].T + bias  -> [8, 1024]
            hc0_ps = PSM.tile([J, 1024], mdt.float32, tag="smallps")
            for fkt in range(FKT):
                wt = WS.tile([128, 1024], mdt.bfloat16, tag="ihc_t")
                nc.sync.dma_start(wt[:], ihcWT[fkt * 128:(fkt + 1) * 128, :])
                for ch in range(2):
                    nc.tensor.matmul(hc0_ps[:, ch * 512:(ch + 1) * 512],
                                     meanT[:, fkt * J:(fkt + 1) * J],
                                     wt[:, ch * 512:(ch + 1) * 512],
                                     start=(fkt == 0), stop=False)
            for ch in range(2):
                nc.tensor.matmul(hc0_ps[:, ch * 512:(ch + 1) * 512],
                                 ones_row[:, 0:J],
                                 ihcB_sb[:, ch * 512:(ch + 1) * 512],
                                 start=False, stop=True)

            nc.vector.tensor_copy(S_sb[1][:], hc0_ps[:, 512:1024])
            h0_bf = PA.tile([J, H], mdt.bfloat16, tag="h0bf")
            nc.vector.tensor_copy(h0_bf[:], hc0_ps[:, 0:512])

            hT0_ps = PSM.tile([128, HKT * J], mdt.bfloat16, tag="smallps")
            for kt in range(HKT):
                nc.tensor.transpose(hT0_ps[:, kt * J:(kt + 1) * J],
                                    h0_bf[:, kt * 128:(kt + 1) * 128], eye8[:])
            nc.vector.tensor_copy(hT0_sb[:], hT0_ps[:])

            # we_proj -> DRAM (reread per step at partition base 0)
            for mt, (m0, mn) in enumerate([(0, 128), (128, 32)]):
                wp_ps = PSB.tile([128, G4], mdt.float32, tag="bigps")
                for ch in range(NCH):
                    c0, c1 = ch * 512, (ch + 1) * 512
                    for kt in range(HKT):
                        nc.tensor.matmul(
                            wp_ps[0:mn, c0:c1],
                            weT_sb[:, kt * TOK + m0: kt * TOK + m0 + mn],
                            WiheT_sb[:, kt * G4 + c0: kt * G4 + c1],
                            start=(kt == 0), stop=False)
                    nc.tensor.matmul(
                        wp_ps[0:mn, c0:c1],
                        ones_row[:, m0:m0 + mn] if mt == 0 else ones_row[:, 0:mn],
                        wihe_brow[:, c0:c1],
                        start=False, stop=True)
                wp_bf = PA.tile([128, G4], mdt.bfloat16, tag="wpbf")
                nc.vector.tensor_copy(wp_bf[0:mn, :], wp_ps[0:mn, :])
                nc.sync.dma_start(weproj_dram[m0:m0 + mn, :], wp_bf[0:mn, :])

        # --- phase A2: a1 and fproj ---
        with tc.tile_pool(name="phA2", bufs=1) as PA2, \
             tc.tile_pool(name="psA1", bufs=1, space="PSUM") as PSA1:

            WaT_sb = PA2.tile([128, FKT * H], mdt.bfloat16, tag="WaT")
            nc.sync.dma_start(WaT_sb[:].rearrange("p (k h) -> p k h", k=FKT),
                              WaT.rearrange("(k p) h -> p k h", p=128))

            WihFT_sb = PA2.tile([128, FKT * G4], mdt.bfloat16, tag="WihFT")
            nc.sync.dma_start(WihFT_sb[:].rearrange("p (k g) -> p k g", k=FKT),
                              WihFT.rearrange("(k p) g -> p k g", p=128))

            # a1T[h', (b l)] = sum_f Wa[h', f] feat[(b,l), f] + (ba + bu)
            for kt2 in range(HKT):
                a1_ps = PSA1.tile([128, BL], mdt.float32, tag="a1ps")
                for fkt in range(FKT):
                    nc.tensor.matmul(
                        a1_ps[:],
                        WaT_sb[:, fkt * H + kt2 * 128: fkt * H + (kt2 + 1) * 128],
                        featT_sb[:, fkt * BL:(fkt + 1) * BL],
                        start=(fkt == 0), stop=False)
                nc.tensor.matmul(a1_ps[:],
                                 baR_sb[:, kt2 * 128:(kt2 + 1) * 128], ones392[:],
                                 start=False, stop=True)
                nc.vector.tensor_copy(a1T_sb[:, kt2 * BL:(kt2 + 1) * BL], a1_ps[:])

            # fproj[(b,l), g] = sum_f feat[(b,l), f] W_ihF[g, f]
            for q in range(4):
                fp_ps = PSB.tile([128, G4], mdt.float32, tag="bigps")
                for fkt in range(FKT):
                    for ch in range(NCH):
                        nc.tensor.matmul(
                            fp_ps[0:98, ch * 512:(ch + 1) * 512],
                            featT_sb[:, fkt * BL + q * 98: fkt * BL + (q + 1) * 98],
                            WihFT_sb[:, fkt * G4 + ch * 512: fkt * G4 + (ch + 1) * 512],
                            start=(fkt == 0), stop=(fkt == FKT - 1))
                nc.vector.tensor_copy(fproj_sb[q][0:98, :], fp_ps[0:98, :])

    # ---------------- phase B: 20 recurrent steps ----------------
    with tc.tile_pool(name="phB", bufs=2) as PB, \
         tc.tile_pool(name="weprj", bufs=3) as WP, \
         tc.tile_pool(name="psB_g", bufs=1, space="PSUM") as PSG, \
         tc.tile_pool(name="psB_s", bufs=1, space="PSUM") as PSS:

        for t in range(T):
            def hT_prev(kt, _t=t):
                if _t == 0:
                    return hT0_sb[:, kt * J:(kt + 1) * J]
                return HT[:, kt * TOK + (_t - 1) * J: kt * TOK + (_t - 1) * J + J]

            wep = WP.tile([J, G4], mdt.bfloat16, tag="wep")
            nc.sync.dma_start(wep[:], weproj_dram[t * J:(t + 1) * J, :])

            # a2T = Ua_h @ H'  (transposed layout [128, (kt2)(j)])
            a2_ps = PSS.tile([128, HKT * J], mdt.float32, tag="a2ps")
            for kt2 in range(HKT):
                for kt in range(HKT):
                    nc.tensor.matmul(
                        a2_ps[:, kt2 * J:(kt2 + 1) * J],
                        UaT_sb[:, kt * H + kt2 * 128: kt * H + (kt2 + 1) * 128],
                        hT_prev(kt),
                        start=(kt == 0), stop=(kt == HKT - 1))
            a2_bf = PB.tile([128, HKT * J], mdt.bfloat16, tag="a2bf")
            nc.vector.tensor_copy(a2_bf[:], a2_ps[:])

            # s = tanh(a1 + a2)
            s_in = PB.tile([128, HKT * BL], mdt.bfloat16, tag="s_in")
            nc.vector.tensor_tensor(
                s_in[:].rearrange("p (k b l) -> p k b l", k=HKT, b=J),
                a1T_sb[:].rearrange("p (k b l) -> p k b l", k=HKT, b=J),
                a2_bf[:].rearrange("p (k b) -> p k b", k=HKT).unsqueeze(3)
                    .broadcast_to([128, HKT, J, L]),
                op=ALU.add)
            sT = PB.tile([128, HKT * BL], mdt.bfloat16, tag="sT")
            nc.scalar.activation(sT[:], s_in[:], AF.Tanh)

            # score rows (replicated over partitions) = va . s
            sc_ps = PSS.tile([J, BL], mdt.float32, tag="scps")
            for kt in range(HKT):
                nc.tensor.matmul(sc_ps[:],
                                 vaRep_sb[:, kt * J:(kt + 1) * J],
                                 sT[:, kt * BL:(kt + 1) * BL],
                                 start=(kt == 0), stop=(kt == HKT - 1))

            # masked softmax over own group of 49
            smask = PB.tile([J, BL], mdt.float32, tag="smask")
            nc.vector.tensor_tensor(smask[:], sc_ps[:], mask_sb[:], op=ALU.add)
            w_un = PB.tile([J, BL], mdt.float32, tag="w_un")
            den = PB.tile([J, 1], mdt.float32, tag="den")
            nc.scalar.activation(w_un[:], smask[:], AF.Exp, accum_out=den[:])
            rden = PB.tile([J, 1], mdt.float32, tag="rden")
            nc.vector.reciprocal(rden[:], den[:])
            w_bf = PB.tile([J, BL], mdt.bfloat16, tag="w_bf")
            nc.vector.tensor_scalar_mul(w_bf[:], w_un[:], rden[:])

            # ws output: reduce groups (off-group entries are 0) then normalize
            nc.vector.tensor_reduce(
                ws_sb[:, t * L:(t + 1) * L],
                w_un[:].rearrange("p (g l) -> p l g", g=J, l=L),
                axis=AX.X, op=ALU.add)
            nc.vector.tensor_scalar_mul(ws_sb[:, t * L:(t + 1) * L],
                                        ws_sb[:, t * L:(t + 1) * L], rden[:])

            # wT: transpose w for the block-diagonal context matmul
            wT_ps = PSS.tile([128, 4 * J], mdt.bfloat16, tag="wTps")
            for q in range(4):
                nc.tensor.transpose(wT_ps[0:98, q * J:(q + 1) * J],
                                    w_bf[:, q * 98:(q + 1) * 98], eye8[:])
            wT_sb = PB.tile([128, 4 * J], mdt.bfloat16, tag="wT_sb")
            nc.vector.tensor_copy(wT_sb[0:98, :], wT_ps[0:98, :])

            # gates PSUM accumulation: H'@(0.5*W_big) + blockdiag-ctx + we_proj
            g_ps = PSG.tile([J, G4], mdt.float32, tag="gates")
            for ch in range(NCH):
                c0, c1 = ch * 512, (ch + 1) * 512
                for kt in range(HKT):
                    nc.tensor.matmul(g_ps[:, c0:c1], hT_prev(kt),
                                     WbigT_sb[:, kt * G4 + c0: kt * G4 + c1],
                                     start=(kt == 0), stop=False)
                for q in range(4):
                    nc.tensor.matmul(g_ps[:, c0:c1],
                                     wT_sb[0:98, q * J:(q + 1) * J],
                                     fproj_sb[q][0:98, c0:c1],
                                     start=False, stop=False)
                nc.tensor.matmul(g_ps[:, c0:c1], eye8[:], wep[:, c0:c1],
                                 start=False, stop=True)

            # LSTM tail (gates order [i, f, o, g]); S = 2c, H' = 2h
            tifo = PB.tile([J, 1536], mdt.float32, tag="tifo")
            nc.scalar.activation(tifo[:], g_ps[:, 0:1536], AF.Tanh, scale=0.5)
            tg = PB.tile([J, H], mdt.float32, tag="tg")
            nc.scalar.activation(tg[:], g_ps[:, 1536:2048], AF.Tanh)
            Asb = PB.tile([J, H], mdt.float32, tag="Asb")
            nc.vector.scalar_tensor_tensor(Asb[:], tifo[:, 512:1024], 1.0,
                                           S_sb[(t + 1) % 2][:],
                                           op0=ALU.add, op1=ALU.mult)
            Bsb = PB.tile([J, H], mdt.float32, tag="Bsb")
            nc.vector.scalar_tensor_tensor(Bsb[:], tifo[:, 0:512], 1.0, tg[:],
                                           op0=ALU.add, op1=ALU.mult)
            nc.vector.scalar_tensor_tensor(S_sb[t % 2][:], Asb[:], 0.5, Bsb[:],
                                           op0=ALU.mult, op1=ALU.add)
            tc_sb = PB.tile([J, H], mdt.float32, tag="tc_sb")
            nc.scalar.activation(tc_sb[:], S_sb[t % 2][:], AF.Tanh, scale=0.5)
            h_bf = PB.tile([J, H], mdt.bfloat16, tag="h_bf")
            nc.vector.scalar_tensor_tensor(h_bf[:], tifo[:, 1024:1536], 1.0,
                                           tc_sb[:], op0=ALU.add, op1=ALU.mult)

            hT_ps = PSS.tile([128, HKT * J], mdt.bfloat16, tag="hTps")
            for kt in range(HKT):
                nc.tensor.transpose(hT_ps[:, kt * J:(kt + 1) * J],
                                    h_bf[:, kt * 128:(kt + 1) * 128], eye8[:])
            nc.vector.tensor_copy(
                HT[:].rearrange("p (k m) -> p k m", k=HKT)[:, :, t * J:(t + 1) * J],
                hT_ps[:].rearrange("p (k j) -> p k j", k=HKT))

        nc.sync.dma_start(wsD.rearrange("j t l -> j (t l)"), ws_sb[:])

    # ---------------- phase C: logits ----------------
    with tc.tile_pool(name="phC", bufs=4) as PC, \
         tc.tile_pool(name="fcb", bufs=4) as PCB, \
         tc.tile_pool(name="psC", bufs=4, space="PSUM") as PSC:
        for v, (v0, vn) in enumerate(_chunks_v()):
            ft = fcw_tiles[v]
            fcb_t = PCB.tile([1, 512], mdt.bfloat16, tag="fcb_t")
            nc.sync.dma_start(fcb_t[:, 0:vn], fcB[:, v0:v0 + vn])
            for mt, (m0, mn) in enumerate([(0, 128), (128, 32)]):
                ops = PSC.tile([128, 512], mdt.float32, tag="ops")
                for kt in range(HKT):
                    nc.tensor.matmul(ops[0:mn, 0:vn],
                                     HT[:, kt * TOK + m0: kt * TOK + m0 + mn],
                                     ft[:, kt, 0:vn],
                                     start=(kt == 0), stop=False)
                nc.tensor.matmul(ops[0:mn, 0:vn],
                                 ones_row[:, 0:mn],
                                 fcb_t[:, 0:vn], start=False, stop=True)
                stage = PC.tile([128, 512], mdt.float32, tag="stage")
                nc.vector.tensor_copy(stage[0:mn, 0:vn], ops[0:mn, 0:vn])
                t0, tn = m0 // J, mn // J
                dst = outsD[:, t0:t0 + tn, v0:v0 + vn].transpose([1, 0, 2])
                nc.sync.dma_start(dst, stage[0:mn, 0:vn])


def make_nc():
    nc = bacc.Bacc("TRN2", target_bir_lowering=False, debug=False,
                   num_devices=NCORES)
    d = {}

    def inp(name, shape, dtype=mdt.bfloat16):
        d[name] = nc.dram_tensor(name, shape, dtype, kind="ExternalInput").ap()

    inp("featT", [F, BL])
    inp("weT", [E, TOK])
    inp("WaT", [F, H])
    inp("WihFT", [F, G4])
    inp("WiheT", [E + 1, G4])
    inp("WbigT", [H, G4])
    inp("UaT", [H, H])
    inp("ihcWT", [F, 1024])
    inp("ihcB", [1, 1024])
    inp("baR", [1, H])
    inp("vaRep", [H, J])
    inp("fcWT", [H, V])
    inp("fcB", [1, V])
    inp("maskD", [J, BL], mdt.float32)
    d["outs"] = nc.dram_tensor("outs", [J, T, V], mdt.float32,
                               kind="ExternalOutput").ap()
    d["ws"] = nc.dram_tensor("ws", [J, T, L], mdt.float32,
                             kind="ExternalOutput").ap()

    with tile.TileContext(nc) as tc:
        with ExitStack() as ctx:
            build_ir(ctx, tc, d)
    nc.compile()
    return nc


def prep_inputs(captions, features, emb, W_ih, W_hh, b_ih, b_hh, fc_W, fc_b,
                Wa, ba, Ua, bu, va, bv, ih_W, ih_b, ic_W, ic_b):
    """Host-side sharding/layout prep. Returns list of per-core input dicts."""
    captions = np.asarray(captions).astype(np.int64)
    features = np.asarray(features, np.float32)
    emb = np.asarray(emb, np.float32)

    perm = np.concatenate([np.arange(0, 1024), np.arange(1536, 2048),
                           np.arange(1024, 1536)])   # [i, f, o, g]
    W_ih = np.asarray(W_ih, np.float32)[perm]
    bias_g = (np.asarray(b_ih, np.float32) + np.asarray(b_hh, np.float32))[perm]
    W_hh_p = 0.5 * np.asarray(W_hh, np.float32)[perm]
    Ua_h = 0.5 * np.asarray(Ua, np.float32)
    ba_h = np.asarray(ba, np.float32) + np.asarray(bu, np.float32)
    ihc_W = np.concatenate([2.0 * np.asarray(ih_W, np.float32),
                            2.0 * np.asarray(ic_W, np.float32)], axis=0)
    ihc_b = np.concatenate([2.0 * np.asarray(ih_b, np.float32),
                            2.0 * np.asarray(ic_b, np.float32)])
    fc_Wh = 0.5 * np.asarray(fc_W, np.float32)
    va_v = np.asarray(va, np.float32)[0]

    WiheT = np.concatenate([np.ascontiguousarray(W_ih[:, :E].T),
                            bias_g[None, :]], axis=0)

    common = {
        "WaT": np.ascontiguousarray(np.asarray(Wa, np.float32).T).astype(BF16),
        "WihFT": np.ascontiguousarray(W_ih[:, E:].T).astype(BF16),
        "WiheT": np.ascontiguousarray(WiheT).astype(BF16),
        "WbigT": np.ascontiguousarray(W_hh_p.T).astype(BF16),
        "UaT": np.ascontiguousarray(Ua_h.T).astype(BF16),
        "ihcWT": np.ascontiguousarray(ihc_W.T).astype(BF16),
        "ihcB": ihc_b[None, :].astype(BF16),
        "baR": ba_h[None, :].astype(BF16),
        "vaRep": np.ascontiguousarray(np.repeat(va_v[:, None], J, 1)).astype(BF16),
        "fcWT": np.ascontiguousarray(fc_Wh.T).astype(BF16),
        "fcB": np.asarray(fc_b, np.float32)[None, :].astype(BF16),
        "maskD": np.where(
            (np.arange(BL)[None, :] // L) == np.arange(J)[:, None],
            0.0, -30000.0).astype(np.float32),
    }

    embed = emb[captions]                                   # [B, T, E]
    word_embeds = np.concatenate(
        [np.zeros((B, 1, E), np.float32), embed[:, :-1]], axis=1)

    in_maps = []
    for c in range(NCORES):
        bs = slice(c * J, (c + 1) * J)
        m = dict(common)
        m["featT"] = np.ascontiguousarray(
            features[bs].reshape(BL, F).T).astype(BF16)
        m["weT"] = np.ascontiguousarray(
            word_embeds[bs].transpose(2, 1, 0).reshape(E, TOK)).astype(BF16)
        in_maps.append(m)
    return in_maps


_NC_CACHE = None


def kernel(**inputs):
    global _NC_CACHE
    from concourse.bass_utils import run_bass_kernel_spmd
    if _NC_CACHE is None:
        _NC_CACHE = make_nc()
    nc = _NC_CACHE
    in_maps = prep_inputs(**inputs)
    res = run_bass_kernel_spmd(nc, in_maps, core_ids=list(range(NCORES)))
    outs = np.concatenate([res.results[c]["outs"] for c in range(NCORES)], 0)
    ws = np.concatenate([res.results[c]["ws"] for c in range(NCORES)], 0)
    return outs, ws
